# revision 33
# baseline (speedup 1.0000x reference)
"""EnhancedAdaptiveLoRAPooling fused kernel for 8x Trainium2 NeuronCores.

Strategy (data-parallel over batch):
  - hidden_states [8, 4096, 768] is sharded by batch element: core i gets
    x_i [4096, 768], pre-transposed AND bf16-cast on host to a p-major
    layout xT2 [128, NTT*NCH*TT] so every token tile is one DMA with 128
    contiguous 12 KiB runs.  bf16 transport halves the HBM traffic of this
    memory-bound kernel; worst-case quantization error is ~2*2^-9 of
    max|y|, far inside the 2e-2 gate.
  - All routing math is computed on-device, replicated on every core.
    The similarity MLP runs TRANSPOSED (units on partitions) so each layer
    is a chain of 16-column matmuls + one ACT relu with a per-partition
    bias: no PE<->ACT transpose ping-pong, and nn_sim lands directly in
    row form for the top-3 selection.  W1/W2/comb travel as fp8e4.
  - The two LoRA branches (current-task + pooled) are fused into a single
    rank-16 LoRA and the routing combiner G2 is folded into the B side:
       M_c = G2 @ B_comb_c   ->   y = x + M.T @ (Abank @ x)
  - Per 1024-token tile: 12 bf16 v-matmuls (all-task A bank, independent
    of routing), 12 bf16 expansion matmuls, and 12 PSUM drains split
    across DVE (drain+residual add), ACT (copy) + DVE bf16 2x add, and
    ACT + gpsimd add.
  - One input queue (sync ring): MLP consts, then x0, then the remaining
    consts, then x1-x3 (FIFO ordering gives the prologue its weights
    ~5 us before the first x tile is needed).  y-out on the gpsimd ring.
"""

import numpy as np

B, S, H = 8, 4096, 768
N_TASKS, R = 16, 8
SCALING = 2.0
NCORES = 8
TPC = (B * S) // NCORES          # tokens per core = 4096
TT = 1024                        # token tile
HTT = TT // 2                    # PSUM-bank-sized half tile (512 f32)
NTT = TPC // TT                  # token tiles per core
NCH = H // 128                   # 6 hidden chunks
NR = N_TASKS * R                 # 128 = (task, rank) pairs

# bigblob layout (f32 column units): MLP + similarity consts
_F32C = 4 + 2 + 1                        # b1T | b2T | b3T
_BF16C = 128 + 1 + 48 + 3                # W3T | W4T | teT | curT
_FP8C = 48 + 1536 + 256                  # combT | W1T | W2T
FBIG = _F32C + _BF16C + _FP8C
FB3 = 16 + 1 + 64                        # blob3: M8 | onehot | I128(bf16)
F2 = 768 + 128 + 16 + 768 + 16 + 128 + 1

_PROGRAM = None


def _build_program():
    from contextlib import ExitStack

    import concourse.bass as bass  # noqa: F401
    import concourse.tile as tile
    from concourse import bacc, mybir

    f32 = mybir.dt.float32
    bf16 = mybir.dt.bfloat16
    fp8 = mybir.dt.float8e4
    AF = mybir.ActivationFunctionType
    OP = mybir.AluOpType
    AX = mybir.AxisListType

    nc = bacc.Bacc("TRN2", target_bir_lowering=False, debug=False)

    def din(name, shape, dt=None):
        return nc.dram_tensor(name, shape, dt or f32, kind="ExternalInput").ap()

    xT2 = din("xT2", [128, NTT * NCH * TT], bf16)     # per-core shard, p-major
    laGTb = din("laGTb", [128, NCH * 128], bf16)      # A bank, transposed
    lbgb = din("lbgb", [128, H], bf16)                # B bank, (task,rank) rows
    bigblob = din("bigblob", [128, FBIG])
    blob3 = din("blob3", [128, FB3])
    blob2 = din("blob2", [16, F2])

    yT2 = nc.dram_tensor("yT2", [128, NTT * NCH * TT], bf16,
                         kind="ExternalOutput").ap()

    with tile.TileContext(nc) as tc:
        with ExitStack() as ctx:
            const = ctx.enter_context(tc.tile_pool(name="const", bufs=1))
            pers = ctx.enter_context(tc.tile_pool(name="pers", bufs=1))
            xp = ctx.enter_context(tc.tile_pool(name="xp", bufs=4))

            # ---- one input queue (sync ring): blob2+A bank, then x0 (so
            # the v0 matmuls run while the MLP consts stream), then the
            # rest of the consts, then x1-x3 ----
            blob2_sb = const.tile([16, F2], f32, name="blob2_sb")
            nc.sync.dma_start(out=blob2_sb, in_=blob2)
            laGT_sb = const.tile([128, NCH, 128], bf16, name="laGT_sb")
            nc.sync.dma_start(out=laGT_sb,
                              in_=laGTb.rearrange("p (c j) -> p c j", c=NCH))

            def xdma(it):
                xt = xp.tile([128, NCH, TT], bf16, tag="xt", name=f"xt{it}")
                nc.sync.dma_start(
                    out=xt,
                    in_=xT2[:, it * NCH * TT:(it + 1) * NCH * TT].rearrange(
                        "p (c t) -> p c t", c=NCH))
                return xt

            xts = [xdma(0)]
            bigblob_sb = const.tile([128, FBIG], f32, name="bigblob_sb")
            nc.sync.dma_start(out=bigblob_sb, in_=bigblob)
            blob3_sb = const.tile([128, FB3], f32, name="blob3_sb")
            nc.sync.dma_start(out=blob3_sb, in_=blob3)
            lbG_sb = const.tile([128, H], bf16, name="lbG_sb")
            nc.sync.dma_start(out=lbG_sb, in_=lbgb)

            for it in range(1, NTT):
                xts.append(xdma(it))

            # ---- const views ----
            def cut1(off, n):
                return bigblob_sb[:, off:off + n]
            b1T_sb = cut1(0, 4)
            b2T_sb = cut1(4, 2)
            b3T_sb = cut1(6, 1)
            bfs = bigblob_sb[:, _F32C:_F32C + _BF16C].bitcast(bf16)
            W3T_sb = bfs[:, 0:256].rearrange("p (c j) -> p c j", c=2)
            W4T_sb = bfs[:, 256:257]
            teT_sb = bfs[:, 258:354].rearrange("p (c j) -> p c j", c=6)
            curT_sb = bfs[:, 354:360].rearrange("p (c j) -> p c j", c=6)
            f8s = bigblob_sb[:, _F32C + _BF16C:FBIG].bitcast(fp8)
            combT_sb = f8s[:, 0:192].rearrange("p (c j) -> p c j", c=12)
            W1T_sb = f8s[:, 192:6336].rearrange("p (c j) -> p c j", c=12)
            W2T_sb = f8s[:, 6336:7360].rearrange("p (c j) -> p c j", c=4)

            M8_sb = blob3_sb[:, 0:16]
            oh_sb = blob3_sb[:, 16:17]
            I128_sb = blob3_sb[:, 17:81].bitcast(bf16)

            o = [0]
            def cut2(n, rows=16):
                off = o[0]; o[0] += n
                return blob2_sb[:rows, off:off + n]
            te_row_sb = cut2(768)
            E16_sb = cut2(128)
            ident_sb = cut2(16)
            cur_row_sb = cut2(768, rows=1)
            ones16_sb = cut2(16, rows=1)
            ones128_sb = cut2(128, rows=1)
            b4_sb = cut2(1, rows=1)

            # ---- vT infrastructure ----
            vp = ctx.enter_context(tc.tile_pool(name="vp", bufs=2, space="PSUM"))
            vsb = ctx.enter_context(tc.tile_pool(name="vsb", bufs=8))
            v_sbs = {}

            def emit_vT(it, h):
                v_ps = vp.tile([128, HTT], f32, tag="v", name="v_ps")
                for c in range(NCH):
                    nc.tensor.matmul(v_ps, lhsT=laGT_sb[:, c, :],
                                     rhs=xts[it][:, c, h * HTT:(h + 1) * HTT],
                                     start=(c == 0), stop=(c == NCH - 1))
                v_sb = vsb.tile([128, HTT], bf16, tag="v_sb", name=f"v{it}_{h}")
                nc.scalar.copy(v_sb, v_ps)
                v_sbs[(it, h)] = v_sb

            # ================= routing prologue (replicated) =================
            pro = ExitStack()
            pp = pro.enter_context(tc.tile_pool(name="pp", bufs=3, space="PSUM"))

            # PE warm-up: ~4us of dummy matmuls on a memset tile while the
            # consts stream in, so the HAM clock-gate releases (1.2->2.4GHz)
            # right as the real prologue work arrives.
            warm = pers.tile([128, HTT], bf16)
            nc.vector.memset(warm, 0.0)
            for _ in range(10):
                wm_ps = pp.tile([16, HTT], f32, tag="pp")
                nc.tensor.matmul(wm_ps, lhsT=warm[:, 0:16], rhs=warm,
                                 start=True, stop=True)

            # norms (ACT) — need only blob2
            scr_te = pers.tile([16, H], f32)
            te2 = pers.tile([16, 1], f32)
            nc.scalar.activation(scr_te, te_row_sb, AF.Square, accum_out=te2)
            scr_cur = pers.tile([1, H], f32)
            cur2 = pers.tile([1, 1], f32)
            nc.scalar.activation(scr_cur, cur_row_sb, AF.Square, accum_out=cur2)
            t2r_ps = pp.tile([1, 16], f32, tag="pp")
            nc.tensor.transpose(t2r_ps, te2, ident_sb)
            te2_row = pers.tile([1, 16], f32)
            nc.scalar.copy(te2_row, t2r_ps)

            # v0 ahead of anything bigblob-dependent on the PE queue
            emit_vT(0, 0)
            emit_vT(0, 1)

            # dots_row[0, n] = te[n] . cur  (row-oriented)
            dots_ps = pp.tile([1, 16], f32, tag="pp")
            for c in range(NCH):
                nc.tensor.matmul(dots_ps, lhsT=curT_sb[:, c, :], rhs=teT_sb[:, c, :],
                                 start=(c == 0), stop=(c == NCH - 1))
            dots_row = pers.tile([1, 16], f32)
            nc.scalar.copy(dots_row, dots_ps)

            # ---- similarity MLP, transposed: units on partitions ----
            h1T = pers.tile([128, 4, 16], fp8)
            for u in range(4):
                z_ps = pp.tile([128, 16], f32, tag="pp")
                for k in range(12):
                    nc.tensor.matmul(z_ps, lhsT=W1T_sb[:, k, u * 128:(u + 1) * 128],
                                     rhs=combT_sb[:, k, :],
                                     start=(k == 0), stop=(k == 11))
                nc.scalar.activation(h1T[:, u, :], z_ps, AF.Relu,
                                     bias=b1T_sb[:, u:u + 1])
            h2T = pers.tile([128, 2, 16], bf16)
            for u in range(2):
                z_ps = pp.tile([128, 16], f32, tag="pp")
                for k in range(4):
                    nc.tensor.matmul(z_ps, lhsT=W2T_sb[:, k, u * 128:(u + 1) * 128],
                                     rhs=h1T[:, k, :],
                                     start=(k == 0), stop=(k == 3))
                nc.scalar.activation(h2T[:, u, :], z_ps, AF.Relu,
                                     bias=b2T_sb[:, u:u + 1])
            z3_ps = pp.tile([128, 16], f32, tag="pp")
            for k in range(2):
                nc.tensor.matmul(z3_ps, lhsT=W3T_sb[:, k, :], rhs=h2T[:, k, :],
                                 start=(k == 0), stop=(k == 1))
            h3T = pers.tile([128, 16], bf16)
            nc.scalar.activation(h3T, z3_ps, AF.Relu, bias=b3T_sb)
            z4_ps = pp.tile([1, 16], f32, tag="pp")
            nc.tensor.matmul(z4_ps, lhsT=W4T_sb, rhs=h3T, start=True, stop=True)
            nn_row = pers.tile([1, 16], f32)
            nc.scalar.activation(nn_row, z4_ps, AF.Sigmoid, bias=b4_sb)

            # cos / euclid (row form, DVE+ACT)
            emb_n = pers.tile([1, 16], f32)
            nc.scalar.sqrt(emb_n, te2_row)
            curn = pers.tile([1, 1], f32)
            nc.scalar.sqrt(curn, cur2)
            den = pers.tile([1, 16], f32)
            nc.vector.tensor_scalar(den, in0=emb_n, scalar1=curn, scalar2=1e-8,
                                    op0=OP.mult, op1=OP.max)
            rden = pers.tile([1, 16], f32)
            nc.vector.reciprocal(rden, den)
            cos = pers.tile([1, 16], f32)
            nc.vector.tensor_mul(cos, dots_row, rden)
            e2 = pers.tile([1, 16], f32)
            nc.vector.scalar_tensor_tensor(e2, in0=dots_row, scalar=-2.0, in1=te2_row,
                                           op0=OP.mult, op1=OP.add)
            nc.vector.tensor_scalar(e2, in0=e2, scalar1=cur2, scalar2=0.0,
                                    op0=OP.add, op1=OP.max)
            eu = pers.tile([1, 16], f32)
            nc.scalar.sqrt(eu, e2)
            eup1 = pers.tile([1, 16], f32)
            nc.scalar.add(eup1, eu, 1.0)
            es = pers.tile([1, 16], f32)
            nc.vector.reciprocal(es, eup1)

            sims_row = pers.tile([1, 16], f32)
            nc.vector.scalar_tensor_tensor(sims_row, in0=cos, scalar=0.4 / 0.3, in1=es,
                                           op0=OP.mult, op1=OP.add)
            nc.vector.tensor_add(sims_row, sims_row, nn_row)
            nc.vector.tensor_scalar_mul(sims_row, sims_row, 0.3)

            # ---- top-3 threshold (DVE; overlaps vT on PE) ----
            m1 = pers.tile([1, 1], f32)
            nc.vector.reduce_max(m1, sims_row, axis=AX.X)
            msk = pers.tile([1, 16], f32)
            nc.vector.tensor_scalar(msk, in0=sims_row, scalar1=m1, scalar2=None, op0=OP.is_ge)
            s2 = pers.tile([1, 16], f32)
            nc.vector.scalar_tensor_tensor(s2, in0=msk, scalar=-1e30, in1=sims_row,
                                           op0=OP.mult, op1=OP.add)
            m2 = pers.tile([1, 1], f32)
            nc.vector.reduce_max(m2, s2, axis=AX.X)
            msk2 = pers.tile([1, 16], f32)
            nc.vector.tensor_scalar(msk2, in0=s2, scalar1=m2, scalar2=None, op0=OP.is_ge)
            s3 = pers.tile([1, 16], f32)
            nc.vector.scalar_tensor_tensor(s3, in0=msk2, scalar=-1e30, in1=s2,
                                           op0=OP.mult, op1=OP.add)
            m3 = pers.tile([1, 1], f32)
            nc.vector.reduce_max(m3, s3, axis=AX.X)
            ge3 = pers.tile([1, 16], f32)
            nc.vector.tensor_scalar(ge3, in0=sims_row, scalar1=m3, scalar2=None, op0=OP.is_ge)
            pos = pers.tile([1, 16], f32)
            nc.vector.tensor_scalar(pos, in0=sims_row, scalar1=0.0, scalar2=None, op0=OP.is_gt)
            m12 = pers.tile([1, 16], f32)
            nc.vector.tensor_mul(m12, ge3, pos)
            w_row = pers.tile([1, 16], f32)
            total = pers.tile([1, 1], f32)
            nc.vector.scalar_tensor_tensor(w_row, in0=m12, scalar=1.0, in1=sims_row,
                                           op0=OP.mult, op1=OP.mult, accum_out=total)
            tpos = pers.tile([1, 1], f32)
            nc.vector.tensor_scalar(tpos, in0=total, scalar1=0.0, scalar2=None, op0=OP.is_gt)
            tm1 = pers.tile([1, 1], f32)
            nc.vector.tensor_scalar_add(tm1, total, -1.0)
            safe = pers.tile([1, 1], f32)
            nc.vector.scalar_tensor_tensor(safe, in0=tm1, scalar=tpos, in1=ones16_sb[:, 0:1],
                                           op0=OP.mult, op1=OP.add)
            rinv = pers.tile([1, 1], f32)
            nc.vector.reciprocal(rinv, safe)
            wn_row = pers.tile([1, 16], f32)
            nc.vector.tensor_scalar_mul(wn_row, w_row, rinv)

            # fusion coefficients
            fw = pers.tile([1, 1], f32)
            nc.vector.tensor_scalar(fw, in0=curn, scalar1=0.1, scalar2=0.5,
                                    op0=OP.mult, op1=OP.min)
            cc = pers.tile([1, 2], f32)   # [c2*S | c1*S]
            c2v = pers.tile([1, 1], f32)
            nc.vector.tensor_mul(c2v, fw, tpos)
            nc.vector.tensor_scalar_mul(cc[:, 0:1], c2v, SCALING)
            nc.vector.tensor_scalar(cc[:, 1:2], in0=cc[:, 0:1], scalar1=-1.0, scalar2=SCALING,
                                    op0=OP.mult, op1=OP.add)
            # the whole G2 -> B_comb -> M chain is serial cross-engine
            # hops; its PSUM drains go on DVE so they never queue behind
            # the ACT v-copies.
            ccb_ps = pp.tile([128, 2], f32, tag="pp")
            nc.tensor.matmul(ccb_ps, lhsT=ones128_sb, rhs=cc, start=True, stop=True)
            cc_b = pers.tile([128, 2], f32)
            nc.vector.tensor_copy(cc_b, ccb_ps)

            # wn onto 128 (task,rank) partitions
            wc_ps = pp.tile([16, 1], f32, tag="pp")
            nc.tensor.transpose(wc_ps, wn_row, ident_sb[:1, :1])
            wn_col = pers.tile([16, 1], f32)
            nc.vector.tensor_copy(wn_col, wc_ps)
            we_ps = pp.tile([128, 1], f32, tag="pp")
            nc.tensor.matmul(we_ps, lhsT=E16_sb, rhs=wn_col, start=True, stop=True)
            wn_ext = pers.tile([128, 1], f32)
            nc.vector.tensor_copy(wn_ext, we_ps)
            # selectors: G2 [128,16] (A-side combiner, scaled) + B_comb
            sc_a = pers.tile([128, 16], f32)
            nc.vector.tensor_scalar_mul(sc_a[:, 0:8], M8_sb[:, 0:8], oh_sb)
            nc.vector.tensor_scalar_mul(sc_a[:, 8:16], M8_sb[:, 8:16], wn_ext)
            G2f = pers.tile([128, 16], f32)
            nc.vector.tensor_scalar(G2f[:, 0:8], in0=sc_a[:, 0:8], scalar1=cc_b[:, 1:2],
                                    scalar2=None, op0=OP.mult)
            nc.vector.tensor_scalar(G2f[:, 8:16], in0=sc_a[:, 8:16], scalar1=cc_b[:, 0:1],
                                    scalar2=None, op0=OP.mult)
            G2b = pers.tile([128, 16], bf16)
            nc.vector.tensor_copy(G2b, G2f)
            sc_ab = pers.tile([128, 16], bf16)
            nc.vector.tensor_copy(sc_ab, sc_a)

            bc_ps = pp.tile([16, H], f32, tag="bc", bufs=1)
            nc.tensor.matmul(bc_ps[:, 0:512], lhsT=sc_ab, rhs=lbG_sb[:, 0:512],
                             start=True, stop=True)
            nc.tensor.matmul(bc_ps[:, 512:768], lhsT=sc_ab, rhs=lbG_sb[:, 512:768],
                             start=True, stop=True)
            B_comb = pers.tile([16, H], bf16)
            nc.vector.tensor_copy(B_comb, bc_ps)

            # fold G2 into the B side:  M_c = G2 @ B_comb_c  [128, 768]
            g2t_ps = pp.tile([16, 128], bf16, tag="pp")
            nc.tensor.transpose(g2t_ps, G2b, I128_sb)
            G2T = pers.tile([16, 128], bf16)
            nc.vector.tensor_copy(G2T, g2t_ps)
            M_sb = pers.tile([128, H], bf16)
            for half in range(2):
                m_ps = pp.tile([128, 384], f32, tag="pp")
                nc.tensor.matmul(m_ps, lhsT=G2T,
                                 rhs=B_comb[:, half * 384:(half + 1) * 384],
                                 start=True, stop=True)
                eng = nc.vector if half == 0 else nc.scalar
                (eng.tensor_copy if half == 0 else eng.copy)(
                    M_sb[:, half * 384:(half + 1) * 384], m_ps)

            pro.close()

            # ================= main loop =================
            yT2_r = yT2.rearrange("p (i ct) -> p i ct", i=NTT)
            with (
                tc.tile_pool(name="yp", bufs=2) as yp,
                tc.tile_pool(name="lsb", bufs=4) as lsb,
                tc.tile_pool(name="lsg", bufs=2) as lsg,
                tc.tile_pool(name="lps", bufs=3, space="PSUM") as lps,
            ):
                def tile_body(it):
                    xt = xts[it]
                    yt = yp.tile([128, NCH, TT], bf16, tag="yt", name="yt")
                    for c in range(NCH):
                        # full-width 1024 block: 2 expansion matmuls sharing
                        # one weight load, one 1024-wide drain
                        l_ps = lps.tile([128, TT], f32, tag="lora", name="l_ps")
                        for h in range(2):
                            nc.tensor.matmul(l_ps[:, h * HTT:(h + 1) * HTT],
                                             lhsT=M_sb[:, c * 128:(c + 1) * 128],
                                             rhs=v_sbs[(it, h)],
                                             start=True, stop=True)
                        if c in (1, 2, 3):
                            # DVE drains PSUM + adds residual (1x, PSUM port)
                            nc.vector.tensor_add(yt[:, c, :], xt[:, c, :], l_ps)
                        else:
                            # ACT drains PSUM; slow gpsimd add gets the FIRST
                            # block so it overlaps the rest of the tile.  The
                            # gpsimd blocks use their own SBUF pool so ACT
                            # never WAR-stalls on a slow gpsimd read.
                            pool = lsg if c == 0 else lsb
                            l_sb = pool.tile([128, TT], bf16, tag="l_sb",
                                             name="l_sb")
                            nc.scalar.copy(l_sb, l_ps)
                            eng = nc.gpsimd if c == 0 else nc.vector
                            eng.tensor_add(yt[:, c, :], xt[:, c, :], l_sb)
                    nc.gpsimd.dma_start(out=yT2_r[:, it, :], in_=yt)

                tile_body(0)
                emit_vT(1, 0)
                emit_vT(1, 1)
                tile_body(1)
                emit_vT(2, 0)
                emit_vT(2, 1)
                tile_body(2)
                emit_vT(3, 0)
                emit_vT(3, 1)
                tile_body(3)

    nc.compile()
    return nc


def _get_program():
    global _PROGRAM
    if _PROGRAM is None:
        _PROGRAM = _build_program()
    return _PROGRAM


def _make_in_maps(inputs):
    import ml_dtypes
    bf = ml_dtypes.bfloat16
    f8 = ml_dtypes.float8_e4m3fn

    hs = np.asarray(inputs["hidden_states"], np.float32)
    cur = np.ascontiguousarray(np.asarray(inputs["task_embedding"], np.float32))
    la = np.ascontiguousarray(np.asarray(inputs["loras_a"], np.float32))
    lb = np.ascontiguousarray(np.asarray(inputs["loras_b"], np.float32))
    te = np.ascontiguousarray(np.asarray(inputs["task_embeds"], np.float32))
    W1 = np.asarray(inputs["W1"], np.float32)
    W2 = np.asarray(inputs["W2"], np.float32)
    W3 = np.asarray(inputs["W3"], np.float32)
    W4 = np.asarray(inputs["W4"], np.float32)
    b1 = np.asarray(inputs["b1"], np.float32)
    b2 = np.asarray(inputs["b2"], np.float32)
    b3 = np.asarray(inputs["b3"], np.float32)
    b4 = np.asarray(inputs["b4"], np.float32)
    tid = int(np.asarray(inputs["current_task_id"]))

    idx = np.arange(NR)
    n_idx, r_idx = idx // R, idx % R
    M8 = np.zeros((NR, N_TASKS), np.float32)
    for j in range(N_TASKS):
        M8[:, j] = (r_idx == (j % R)).astype(np.float32)
    E16 = np.zeros((N_TASKS, NR), np.float32)
    E16[n_idx, idx] = 1.0
    onehot_ext = (n_idx == tid).astype(np.float32).reshape(NR, 1)

    def chunkpack(a):
        # [C*128, J] -> [128, C*J] so blob[p, c*J+j] = a[c*128+p, j]
        C = a.shape[0] // 128
        return a.reshape(C, 128, -1).transpose(1, 0, 2).reshape(128, -1)

    def bfpack(a):
        b = np.ascontiguousarray(np.asarray(a, np.float32).astype(bf))
        return b.view(np.float32)

    def f8pack(a):
        b = np.ascontiguousarray(np.asarray(a, np.float32).astype(f8))
        return b.view(np.float32)

    comb = np.concatenate([np.repeat(cur[:, None], N_TASKS, axis=1), te.T], axis=0)
    w4pad = np.concatenate([np.ascontiguousarray(W4.T),
                            np.zeros((128, 1), np.float32)], axis=1)
    bigblob = np.concatenate([
        b1.reshape(4, 128).T,                                # 4   b1T
        b2.reshape(2, 128).T,                                # 2   b2T
        b3.reshape(1, 128).T,                                # 1   b3T
        bfpack(chunkpack(np.ascontiguousarray(W3.T))),       # 128 W3T (bf16)
        bfpack(w4pad),                                       # 1   W4T+pad (bf16)
        bfpack(chunkpack(np.ascontiguousarray(te.T))),       # 48  teT (bf16)
        bfpack(cur.reshape(6, 128).T),                       # 3   curT (bf16)
        f8pack(chunkpack(comb)),                             # 48  combT (fp8)
        f8pack(chunkpack(np.ascontiguousarray(W1.T))),       # 1536 W1T (fp8)
        f8pack(chunkpack(np.ascontiguousarray(W2.T))),       # 256 W2T (fp8)
    ], axis=1).astype(np.float32)
    assert bigblob.shape == (128, FBIG), bigblob.shape

    blob3 = np.concatenate([
        M8,                                                  # 16
        onehot_ext,                                          # 1
        bfpack(np.eye(128, dtype=np.float32)),               # 64  I128 (bf16)
    ], axis=1).astype(np.float32)
    assert blob3.shape == (128, FB3), blob3.shape

    laGTb = np.ascontiguousarray(
        chunkpack(np.ascontiguousarray(la.reshape(NR, H).T)).astype(bf))
    lbgb = np.ascontiguousarray(
        lb.transpose(0, 2, 1).reshape(NR, H).astype(bf))

    def row0(a, n):
        b = np.zeros((16, n), np.float32)
        b[0, :] = a.reshape(-1)
        return b
    blob2 = np.concatenate([
        te,                                                  # 768
        E16,                                                 # 128
        np.eye(16, dtype=np.float32),                        # 16
        row0(cur, 768),
        row0(np.ones(16, np.float32), 16),
        row0(np.ones(NR, np.float32), 128),
        row0(b4, 1),
    ], axis=1).astype(np.float32)
    assert blob2.shape == (16, F2), blob2.shape

    rep = {
        "bigblob": bigblob,
        "blob3": blob3,
        "laGTb": laGTb,
        "lbgb": lbgb,
        "blob2": blob2,
    }

    x2 = hs.reshape(B * S, H)
    in_maps = []
    for i in range(NCORES):
        s = x2[i * TPC:(i + 1) * TPC].reshape(NTT, TT, NCH, 128)
        shard = np.ascontiguousarray(
            s.transpose(3, 0, 2, 1).reshape(128, NTT * NCH * TT).astype(bf))
        in_maps.append({"xT2": shard, **rep})
    return in_maps


def kernel(**inputs):
    from concourse.bass_utils import run_bass_kernel_spmd

    nc = _get_program()
    in_maps = _make_in_maps(inputs)
    res = run_bass_kernel_spmd(nc, in_maps, core_ids=list(range(NCORES)))
    out = np.empty((B * S, H), np.float32)
    for i, r in enumerate(res.results):
        y = np.asarray(r["yT2"]).reshape(128, NTT, NCH, TT)
        out[i * TPC:(i + 1) * TPC] = (
            y.transpose(1, 3, 2, 0).reshape(TPC, H).astype(np.float32))
    return out.reshape(B, S, H)
